# revision 15
# baseline (speedup 1.0000x reference)
"""Trainium2 Bass kernel for AttLayer pooling (B=32, T=2048, D=1024, H=5).

Math (equivalent to reference up to exact cancellation of the softmax
normalization): since |tanh| <= 1, scores s[b,t] are bounded by ||uw||_1, so
exp needs no max-subtraction, and the masked renormalization cancels the
softmax denominator:

    out[b,:] = sum_t x[b,t,:] * g[b,t] / sum_t g[b,t]
    g[b,t]   = exp(s[b,t] + masklog[b,t]),  masklog = 0 or -1e30
    s[b,t]   = sum_h tanh( (x @ W)[b,t,h] + fea[b,t]*Wf[h] + bw[h] ) * uw[h]

Everything is tile-local (no cross-T dependency), so the kernel streams x
in a single pass. Data-parallel across batch: 8 cores x 4 batches each.

Per 128-t tile of x [128, 1024]:
  - PE transposes x chunks (f32r, is_transpose) -> psum -> DVE copy -> xT sbuf
  - scores: psum[5, T_GRP] += W_chunk.T @ xT_chunk   (f32r, N=T_GRP)
            + fea part via K=1 matmul (lhsT=Wf [1,5], rhs=fea row)
  - ACT tanh(scores + bw) -> tanh_b rows 0..4; masklog precomputed in row 5
  - uw matmul per tile: lhsT = tanh_b[:, chunk] [6,128], rhs = uw_aug [6,1]
    -> s' column [128,1] in psum (mask fold: uw_aug[5]=1, row5 = masklog)
  - ACT exp -> g [128,1] f32r
  - num matmuls: psum[1, 1025] += g.T @ [x | ones]  (f32r; col 1024 = den)
Final per batch: out = num * reciprocal(den), DMA out.
"""

import sys

sys.path.insert(0, "/opt/trn_rl_repo")

import numpy as np

import concourse.bass as bass
import concourse.mybir as mybir
import concourse.tile as tile
from concourse import bacc
from concourse.masks import make_identity

F32 = mybir.dt.float32
F32R = mybir.dt.float32r
BF16 = mybir.dt.bfloat16
U8 = mybir.dt.uint8
AF = mybir.ActivationFunctionType

P = 128          # partitions / t-tile size
D = 1024         # feature dim
H = 5            # attention hidden dim
NCHUNK = D // P  # 8 d-chunks per tile


def build_kernel(b_shard: int, T: int, t_grp: int = 512, dma_grp: int = 1024):
    """Build the per-core Bass program.

    b_shard: batches per core; T: sequence length; t_grp: t per compute
    group (multiple of 128, <= 512); dma_grp: t per DMA chunk (multiple of
    t_grp).
    """
    assert t_grp % P == 0 and T % dma_grp == 0 and dma_grp % t_grp == 0
    jg = t_grp // P            # tiles per compute group
    jd = dma_grp // P          # tiles per DMA chunk
    n_dma = T // dma_grp
    grp_per_dma = dma_grp // t_grp

    nc = bacc.Bacc(None)

    x_temp = nc.dram_tensor("x_temp", [b_shard, T, D], F32R, kind="ExternalInput")
    x_fea = nc.dram_tensor("x_fea", [b_shard, T], F32R, kind="ExternalInput")
    mask = nc.dram_tensor("mask", [b_shard, T], U8, kind="ExternalInput")
    W_temp = nc.dram_tensor("W_temp", [D, H], F32, kind="ExternalInput")
    W_fea = nc.dram_tensor("W_fea", [1, H], F32R, kind="ExternalInput")
    bw = nc.dram_tensor("bw", [H], F32, kind="ExternalInput")
    uw = nc.dram_tensor("uw", [H], F32, kind="ExternalInput")
    out = nc.dram_tensor("out", [b_shard, D], F32, kind="ExternalOutput")

    with tile.TileContext(nc) as tc:
        with (
            tc.tile_pool(name="consts", bufs=1) as consts,
            tc.tile_pool(name="xpool", bufs=2) as xpool,
            tc.tile_pool(name="xtpool", bufs=3) as xtpool,
            tc.tile_pool(name="rows", bufs=2) as rows,
            tc.tile_pool(name="small", bufs=2) as small,
            tc.tile_pool(name="tp_ps", bufs=2, space="PSUM") as tp_ps,
            tc.tile_pool(name="sc_ps", bufs=2, space="PSUM") as sc_ps,
            tc.tile_pool(name="g_ps", bufs=1, space="PSUM") as g_ps,
            tc.tile_pool(name="num_ps", bufs=1, space="PSUM") as num_ps,
            tc.tile_pool(name="den_ps", bufs=1, space="PSUM") as den_ps,
        ):
            # ---- constants ----
            # Transposes + scores matmuls run in bf16 (x cast on GpSimd);
            # num matmuls stay f32r on the raw DMA'd x (exact-ish).
            ident = consts.tile([P, P], BF16)
            make_identity(nc, ident[:])
            w_f = consts.tile([P, NCHUNK, H], F32)
            nc.sync.dma_start(w_f[:], W_temp.rearrange("(c p) h -> p c h", p=P))
            w_sb = consts.tile([P, NCHUNK, H], BF16)
            nc.vector.tensor_copy(w_sb[:], w_f[:])
            wf_sb = consts.tile([1, H], F32R)
            nc.sync.dma_start(wf_sb[:], W_fea[:])
            bw_sb = consts.tile([H, 1], F32)
            nc.sync.dma_start(bw_sb[:], bw[:, None])
            # uw_aug = [uw; 1.0]: memset whole tile to 1.0, DMA uw over rows 0..4
            # (engine ops cannot write at base partition 5, DMA can overwrite 0..4)
            uwa_sb = consts.tile([H + 1, 1], F32)
            nc.vector.memset(uwa_sb[:], 1.0)
            nc.sync.dma_start(uwa_sb[:H, :], uw[:, None])
            # ones column for the per-batch den matmul (lhsT, M=1)
            ones_f = consts.tile([P, 1], F32)
            nc.vector.memset(ones_f[:], 1.0)
            ones_sb = consts.tile([P, 1], F32R)
            nc.vector.tensor_copy(ones_sb[:], ones_f[:])

            for b in range(b_shard):
                # ---- per-batch rows ----
                fea_sb = rows.tile([1, T], F32R, tag="fea")
                nc.sync.dma_start(fea_sb[:], x_fea[b : b + 1, :])
                mask_f = rows.tile([1, T], F32, tag="maskf")
                nc.gpsimd.dma_start(mask_f[:], mask[b : b + 1, :])  # u8 -> f32 cast
                masklog = rows.tile([1, T], F32, tag="masklog")
                nc.scalar.activation(
                    masklog[:], mask_f[:], AF.Copy, scale=1.0e30, bias=-1.0e30
                )
                # tanh_b rows 0..4 = tanh(scores) written per group;
                # row 5 = masklog = mask*1e30 - 1e30  (0 or -1e30).
                # SBUF->SBUF DMA: engines can't write at base partition 5.
                tanh_b = rows.tile([H + 1, T], F32, tag="tanhb")
                nc.sync.dma_start(tanh_b[H : H + 1, :], masklog[:])

                nm = num_ps.tile([1, D], F32, tag="num")
                n_tiles = T // P
                g_sb = rows.tile([P, n_tiles], F32R, tag="gsb")

                for di in range(n_dma):
                    x3 = xpool.tile([P, jd, D], F32R, tag="x")
                    nc.sync.dma_start(
                        x3[:],
                        x_temp[b, di * dma_grp : (di + 1) * dma_grp, :].rearrange(
                            "(j p) d -> p j d", p=P
                        ),
                    )
                    for gi in range(grp_per_dma):
                        g = di * grp_per_dma + gi   # group index within batch
                        t0 = g * t_grp
                        # bf16 copy of this group's x for transposes/scores
                        xb = xtpool.tile([P, jg, D], BF16, tag="xb")
                        for j in range(jg):
                            nc.gpsimd.tensor_copy(
                                xb[:, j, :], x3[:, gi * jg + j, :].bitcast(F32)
                            )
                        sc = sc_ps.tile([H, t_grp], F32, tag="sc")
                        # fea part: [5, t_grp] = Wf.T @ fea_row (K=1), starts accum
                        nc.tensor.matmul(
                            sc[:],
                            wf_sb[:],
                            fea_sb[:, t0 : t0 + t_grp],
                            start=True,
                            stop=False,
                        )
                        for c in range(NCHUNK):
                            tp = tp_ps.tile([P, t_grp], BF16, tag="tp")
                            for j in range(jg):
                                nc.tensor.transpose(
                                    tp[:, j * P : (j + 1) * P],
                                    xb[:, j, c * P : (c + 1) * P],
                                    ident[:],
                                )
                            xt = xtpool.tile([P, t_grp], BF16, tag="xt")
                            nc.vector.tensor_copy(xt[:], tp[:])
                            nc.tensor.matmul(
                                sc[:],
                                w_sb[:, c, :],
                                xt[:],
                                start=False,
                                stop=(c == NCHUNK - 1),
                            )
                        # tanh(sc + bw) -> tanh_b rows 0..4
                        nc.scalar.activation(
                            tanh_b[:H, t0 : t0 + t_grp], sc[:], AF.Tanh, bias=bw_sb[:]
                        )
                        # uw matmuls: one [128,1] s' column per tile
                        gp = g_ps.tile([P, jg], F32, tag="g")
                        for j in range(jg):
                            nc.tensor.matmul(
                                gp[:, j : j + 1],
                                tanh_b[:, t0 + j * P : t0 + (j + 1) * P],
                                uwa_sb[:],
                                start=True,
                                stop=True,
                            )
                        nc.scalar.activation(
                            g_sb[:, g * jg : (g + 1) * jg], gp[:], AF.Exp
                        )
                        # num accumulation
                        for j in range(jg):
                            jj = gi * jg + j
                            tt = g * jg + j
                            first = tt == 0
                            last = tt == n_tiles - 1
                            nc.tensor.matmul(
                                nm[:, 0:512],
                                g_sb[:, tt : tt + 1],
                                x3[:, jj, 0:512],
                                start=first,
                                stop=last,
                            )
                            nc.tensor.matmul(
                                nm[:, 512:1024],
                                g_sb[:, tt : tt + 1],
                                x3[:, jj, 512:1024],
                                start=first,
                                stop=last,
                            )

                # den = sum of g over all (t); one matmul + free-dim reduce
                dn = den_ps.tile([1, n_tiles], F32, tag="den")
                nc.tensor.matmul(dn[:], ones_sb[:], g_sb[:], start=True, stop=True)
                den_sb = small.tile([1, 1], F32, tag="densb")
                nc.vector.tensor_reduce(
                    den_sb[:], dn[:], axis=mybir.AxisListType.X, op=mybir.AluOpType.add
                )
                inv = small.tile([1, 1], F32, tag="inv")
                nc.vector.reciprocal(inv[:], den_sb[:])
                o_sb = small.tile([1, D], F32, tag="osb")
                nc.vector.tensor_scalar_mul(o_sb[:], nm[:, :D], inv[:])
                nc.sync.dma_start(out[b : b + 1, :], o_sb[:])

    nc.finalize()
    return nc


_NC_CACHE = {}


def _get_nc(b_shard, T):
    key = (b_shard, T)
    if key not in _NC_CACHE:
        _NC_CACHE[key] = build_kernel(b_shard, T)
    return _NC_CACHE[key]


def kernel(x_temp, x_fea, mask, W_temp, W_fea, bw, uw) -> np.ndarray:
    from concourse.bass_utils import run_bass_kernel_spmd

    B, T, D_ = x_temp.shape
    n_cores = 8
    assert B % n_cores == 0
    bs = B // n_cores

    nc = _get_nc(bs, T)

    x_temp = np.ascontiguousarray(x_temp, dtype=np.float32)
    x_fea = np.ascontiguousarray(x_fea, dtype=np.float32)
    mask_u8 = np.ascontiguousarray(mask).view(np.uint8)
    W_temp = np.ascontiguousarray(W_temp, dtype=np.float32)
    W_fea = np.ascontiguousarray(W_fea, dtype=np.float32)
    bw = np.ascontiguousarray(bw, dtype=np.float32)
    uw = np.ascontiguousarray(uw, dtype=np.float32)

    in_maps = []
    for i in range(n_cores):
        in_maps.append(
            {
                "x_temp": x_temp[i * bs : (i + 1) * bs],
                "x_fea": x_fea[i * bs : (i + 1) * bs],
                "mask": mask_u8[i * bs : (i + 1) * bs],
                "W_temp": W_temp,
                "W_fea": W_fea,
                "bw": bw,
                "uw": uw,
            }
        )

    res = run_bass_kernel_spmd(nc, in_maps, core_ids=list(range(n_cores)))
    return np.concatenate([r["out"] for r in res.results], axis=0)


# revision 18
# speedup vs baseline: 1.5888x; 1.5888x over previous
"""Trainium2 Bass kernel for AttLayer pooling (B=32, T=2048, D=1024, H=5).

Math (equivalent to reference up to exact cancellation of the softmax
normalization): since |tanh| <= 1, scores s[b,t] are bounded by ||uw||_1, so
exp needs no max-subtraction, and the masked renormalization cancels the
softmax denominator:

    out[b,:] = sum_t x[b,t,:] * g[b,t] / sum_t g[b,t]
    g[b,t]   = exp(s[b,t] + masklog[b,t]),  masklog = 0 or -1e30
    s[b,t]   = sum_h tanh( (x @ W)[b,t,h] + fea[b,t]*Wf[h] + bw[h] ) * uw[h]

Everything is tile-local (no cross-T dependency), so the kernel streams x
in a single pass. Data-parallel across batch: 8 cores x 4 batches each.

Per 128-t tile of x [128, 1024]:
  - PE transposes x chunks (f32r, is_transpose) -> psum -> DVE copy -> xT sbuf
  - scores: psum[5, T_GRP] += W_chunk.T @ xT_chunk   (f32r, N=T_GRP)
            + fea part via K=1 matmul (lhsT=Wf [1,5], rhs=fea row)
  - ACT tanh(scores + bw) -> tanh_b rows 0..4; masklog precomputed in row 5
  - uw matmul per tile: lhsT = tanh_b[:, chunk] [6,128], rhs = uw_aug [6,1]
    -> s' column [128,1] in psum (mask fold: uw_aug[5]=1, row5 = masklog)
  - ACT exp -> g [128,1] f32r
  - num matmuls: psum[1, 1025] += g.T @ [x | ones]  (f32r; col 1024 = den)
Final per batch: out = num * reciprocal(den), DMA out.
"""

import sys

sys.path.insert(0, "/opt/trn_rl_repo")

import numpy as np

import concourse.bass as bass
import concourse.mybir as mybir
import concourse.tile as tile
from concourse import bacc
from concourse.masks import make_identity
from concourse import bass_isa

F32 = mybir.dt.float32
F32R = mybir.dt.float32r
BF16 = mybir.dt.bfloat16
U8 = mybir.dt.uint8
AF = mybir.ActivationFunctionType

P = 128          # partitions / t-tile size
D = 1024         # feature dim
H = 5            # attention hidden dim
NCHUNK = D // P  # 8 d-chunks per tile


def build_kernel(b_shard: int, T: int, t_grp: int = 512, dma_grp: int = 1024):
    """Build the per-core Bass program.

    b_shard: batches per core; T: sequence length; t_grp: t per compute
    group (multiple of 128, <= 512); dma_grp: t per DMA chunk (multiple of
    t_grp).
    """
    assert t_grp % P == 0 and T % dma_grp == 0 and dma_grp % t_grp == 0
    jg = t_grp // P            # tiles per compute group
    jd = dma_grp // P          # tiles per DMA chunk
    n_dma = T // dma_grp
    grp_per_dma = dma_grp // t_grp

    nc = bacc.Bacc(None)

    x_temp = nc.dram_tensor("x_temp", [b_shard, T, D], F32R, kind="ExternalInput")
    x_fea = nc.dram_tensor("x_fea", [b_shard, T], F32R, kind="ExternalInput")
    mask = nc.dram_tensor("mask", [b_shard, T], U8, kind="ExternalInput")
    W_temp = nc.dram_tensor("W_temp", [D, H], F32, kind="ExternalInput")
    W_fea = nc.dram_tensor("W_fea", [1, H], F32R, kind="ExternalInput")
    bw = nc.dram_tensor("bw", [H], F32, kind="ExternalInput")
    uw = nc.dram_tensor("uw", [H], F32, kind="ExternalInput")
    out = nc.dram_tensor("out", [b_shard, D], F32, kind="ExternalOutput")

    with tile.TileContext(nc) as tc:
        with (
            tc.tile_pool(name="consts", bufs=1) as consts,
            tc.tile_pool(name="xpool", bufs=2) as xpool,
            tc.tile_pool(name="xtpool", bufs=3) as xtpool,
            tc.tile_pool(name="rows", bufs=2) as rows,
            tc.tile_pool(name="small", bufs=2) as small,
            tc.tile_pool(name="tp_ps", bufs=3, space="PSUM") as tp_ps,
            tc.tile_pool(name="sc_ps", bufs=2, space="PSUM") as sc_ps,
            tc.tile_pool(name="g_ps", bufs=1, space="PSUM") as g_ps,
            tc.tile_pool(name="num_ps", bufs=1, space="PSUM") as num_ps,
        ):
            # ---- constants ----
            # Transposes + scores matmuls run in bf16 (x cast on GpSimd);
            # num matmuls stay f32r on the raw DMA'd x (exact-ish).
            ident = consts.tile([P, P], BF16)
            make_identity(nc, ident[:])
            w_f = consts.tile([P, NCHUNK, H], F32)
            nc.sync.dma_start(w_f[:], W_temp.rearrange("(c p) h -> p c h", p=P))
            w_sb = consts.tile([P, NCHUNK, H], BF16)
            nc.vector.tensor_copy(w_sb[:], w_f[:])
            wf_sb = consts.tile([1, H], F32R)
            nc.sync.dma_start(wf_sb[:], W_fea[:])
            bw_sb = consts.tile([H, 1], F32)
            nc.sync.dma_start(bw_sb[:], bw[:, None])
            # uw_aug = [uw; 1.0]: memset whole tile to 1.0, DMA uw over rows 0..4
            # (engine ops cannot write at base partition 5, DMA can overwrite 0..4)
            uwa_f = consts.tile([H + 1, 2], F32)
            nc.vector.memset(uwa_f[:], 1.0)
            nc.sync.dma_start(uwa_f[:H, 0:1], uw[:, None])
            nc.sync.dma_start(uwa_f[:H, 1:2], uw[:, None])
            uwa_sb = consts.tile([H + 1, 2], F32R)
            nc.vector.tensor_copy(uwa_sb[:], uwa_f[:])

            for b in range(b_shard):
                # ---- per-batch rows ----
                fea_sb = rows.tile([1, T], F32R, tag="fea")
                nc.sync.dma_start(fea_sb[:], x_fea[b : b + 1, :])
                mask_f = rows.tile([1, T], F32, tag="maskf")
                nc.gpsimd.dma_start(mask_f[:], mask[b : b + 1, :])  # u8 -> f32 cast
                masklog = rows.tile([1, T], F32R, tag="masklog")
                nc.scalar.activation(
                    masklog[:], mask_f[:], AF.Copy, scale=1.0e30, bias=-1.0e30
                )
                # tanh_b rows 0..4 = tanh(scores) written per group;
                # row 5 = masklog = mask*1e30 - 1e30  (0 or -1e30).
                # SBUF->SBUF DMA: engines can't write at base partition 5.
                tanh_b = rows.tile([H + 1, T], F32R, tag="tanhb")
                nc.sync.dma_start(tanh_b[H : H + 1, :], masklog[:])

                nm = num_ps.tile([1, D], F32, tag="num")
                n_tiles = T // P
                g_sb = rows.tile([P, n_tiles], F32R, tag="gsb")

                for di in range(n_dma):
                    x3 = xpool.tile([P, jd, D], F32R, tag="x")
                    nc.sync.dma_start(
                        x3[:],
                        x_temp[b, di * dma_grp : (di + 1) * dma_grp, :].rearrange(
                            "(j p) d -> p j d", p=P
                        ),
                    )
                    for gi in range(grp_per_dma):
                        g = di * grp_per_dma + gi   # group index within batch
                        t0 = g * t_grp
                        # bf16 copy of this group's x for transposes/scores
                        xb = xtpool.tile([P, jg, D], BF16, tag="xb")
                        for j in range(jg):
                            src = x3[:, gi * jg + j, :].bitcast(F32)
                            if j % 4 == 3:
                                nc.scalar.copy(xb[:, j, :], src)
                            else:
                                nc.vector.tensor_copy(xb[:, j, :], src)
                        sc = sc_ps.tile([H, t_grp], F32, tag="sc")
                        # fea part: [5, t_grp] = Wf.T @ fea_row (K=1), starts accum
                        nc.tensor.matmul(
                            sc[:],
                            wf_sb[:],
                            fea_sb[:, t0 : t0 + t_grp],
                            start=True,
                            stop=False,
                        )
                        for cp in range(NCHUNK // 2):
                            tp = tp_ps.tile([P, 2, t_grp], BF16, tag="tp")
                            for c2 in range(2):
                                c = cp * 2 + c2
                                for j in range(jg):
                                    nc.tensor.transpose(
                                        tp[:, c2, j * P : (j + 1) * P],
                                        xb[:, j, c * P : (c + 1) * P],
                                        ident[:],
                                    )
                            xt = xtpool.tile([P, 2, t_grp], BF16, tag="xt")
                            nc.vector.tensor_copy(xt[:], tp[:])
                            for c2 in range(2):
                                c = cp * 2 + c2
                                nc.tensor.matmul(
                                    sc[:],
                                    w_sb[:, c, :],
                                    xt[:, c2, :],
                                    start=False,
                                    stop=(c == NCHUNK - 1),
                                )
                        # tanh(sc + bw) -> tanh_b rows 0..4
                        nc.scalar.activation(
                            tanh_b[:H, t0 : t0 + t_grp], sc[:], AF.Tanh, bias=bw_sb[:]
                        )
                        # uw matmuls: one [128,1] s' column per tile
                        gp = g_ps.tile([P, jg, 2], F32, tag="g")
                        for j in range(jg):
                            nc.tensor.matmul(
                                gp[:, j, :],
                                tanh_b[:, t0 + j * P : t0 + (j + 1) * P],
                                uwa_sb[:],
                                start=True,
                                stop=True,
                            )
                        nc.scalar.activation(
                            g_sb[:, g * jg : (g + 1) * jg], gp[:, :, 0], AF.Exp
                        )
                        # num accumulation
                        for j in range(jg):
                            jj = gi * jg + j
                            tt = g * jg + j
                            first = tt == 0
                            last = tt == n_tiles - 1
                            nc.tensor.matmul(
                                nm[:, 0:512],
                                g_sb[:, tt : tt + 1],
                                x3[:, jj, 0:512],
                                start=first,
                                stop=last,
                            )
                            nc.tensor.matmul(
                                nm[:, 512:1024],
                                g_sb[:, tt : tt + 1],
                                x3[:, jj, 512:1024],
                                start=first,
                                stop=last,
                            )

                # den = sum of g: DVE free-reduce then GpSimd partition reduce
                gcs = small.tile([P, 1], F32, tag="gcs")
                nc.vector.tensor_reduce(
                    gcs[:],
                    g_sb[:].bitcast(F32),
                    axis=mybir.AxisListType.X,
                    op=mybir.AluOpType.add,
                )
                den_sb = small.tile([P, 1], F32, tag="densb")
                nc.gpsimd.partition_all_reduce(
                    den_sb[:], gcs[:], channels=P, reduce_op=bass_isa.ReduceOp.add
                )
                inv = small.tile([1, 1], F32, tag="inv")
                nc.vector.reciprocal(inv[:], den_sb[0:1, :])
                o_sb = small.tile([1, D], F32, tag="osb")
                nc.vector.tensor_scalar_mul(o_sb[:], nm[:, :D], inv[:])
                nc.sync.dma_start(out[b : b + 1, :], o_sb[:])

    nc.finalize()
    return nc


_NC_CACHE = {}


def _get_nc(b_shard, T):
    key = (b_shard, T)
    if key not in _NC_CACHE:
        _NC_CACHE[key] = build_kernel(b_shard, T)
    return _NC_CACHE[key]


def kernel(x_temp, x_fea, mask, W_temp, W_fea, bw, uw) -> np.ndarray:
    from concourse.bass_utils import run_bass_kernel_spmd

    B, T, D_ = x_temp.shape
    n_cores = 8
    assert B % n_cores == 0
    bs = B // n_cores

    nc = _get_nc(bs, T)

    x_temp = np.ascontiguousarray(x_temp, dtype=np.float32)
    x_fea = np.ascontiguousarray(x_fea, dtype=np.float32)
    mask_u8 = np.ascontiguousarray(mask).view(np.uint8)
    W_temp = np.ascontiguousarray(W_temp, dtype=np.float32)
    W_fea = np.ascontiguousarray(W_fea, dtype=np.float32)
    bw = np.ascontiguousarray(bw, dtype=np.float32)
    uw = np.ascontiguousarray(uw, dtype=np.float32)

    in_maps = []
    for i in range(n_cores):
        in_maps.append(
            {
                "x_temp": x_temp[i * bs : (i + 1) * bs],
                "x_fea": x_fea[i * bs : (i + 1) * bs],
                "mask": mask_u8[i * bs : (i + 1) * bs],
                "W_temp": W_temp,
                "W_fea": W_fea,
                "bw": bw,
                "uw": uw,
            }
        )

    res = run_bass_kernel_spmd(nc, in_maps, core_ids=list(range(n_cores)))
    return np.concatenate([r["out"] for r in res.results], axis=0)
